# revision 8
# baseline (speedup 1.0000x reference)
"""Trainium2 Bass kernel for nn_CRABLayer (dynamic gated 3x3 conv x2 + residual).

Sharding: 8 cores = (batch b in 0..3) x (image half h in 0..1); each core
computes its (b, 96-row) output slab. The tiny cross-batch gating statistics
(adaptive-avg-pool "gl") are combined with one AllGather per layer.

Layout: per-channel padded rows of width 194 (1 zero col each side), flattened
so a 3x3 conv becomes 9 column-shifted matmuls; shifts differing by 194 are
K-stacked (x in SBUF partitions 0:64, x<<194 in 64:128) into K=128 matmuls.
All conv matmuls run in fp16 (fp32 PSUM accumulation); gating math is fp32.
"""
import numpy as np
import concourse.bass as bass
import concourse.bacc as bacc
import concourse.tile as tile
import concourse.mybir as mybir
from concourse.masks import make_identity

F32 = mybir.dt.float32
F16 = mybir.dt.float16
AF = mybir.ActivationFunctionType
ALU = mybir.AluOpType

NCH = 64
EPS = 1e-5
PW = 194
XSLOTS, YSLOTS = 102, 100
XOFF = 1
XW = XOFF + XSLOTS * PW          # 19789
YW = XOFF + YSLOTS * PW          # 19401
OUTW = 96 * PW                   # 18624
PAIR_DELTAS = [-195, -194, -193]
SINGLE_DELTAS = [193, 194, 195]
L1_LO, L1_HI = 194, 19206        # y1 slots [1, 99)
L2_LO, L2_HI = 388, 19012        # out slots [2, 98)
N_CORES = 8
CONV_BLK = 512

PARAM_SHAPES = dict(
    ceT=(9, 5), aff1T=(NCH, NCH), aff2T=(NCH, NCH), wgT=(NCH, NCH),
    wr=(NCH, NCH), gdT=(5, 9), gd2T=(5, 9), ciT=(16, 16),
    wconv=(NCH, 576), bn=(NCH, 6),
)


def ap_of(t, part0, nparts, col0, dims):
    """Custom AP into a pool tile t: partitions [part0, part0+nparts),
    free pattern dims=[[step, count], ...] starting at column col0."""
    a = t[:]
    w = a.ap[0][0]
    # bounds check (elements)
    lo = hi = 0
    for s, c in dims:
        if s >= 0:
            hi += s * (c - 1)
        else:
            lo += s * (c - 1)
    assert col0 + lo >= 0 and col0 + hi <= w - 1, (col0, dims, w)
    assert 0 <= part0 and part0 + nparts <= a.ap[0][1], (part0, nparts)
    return bass.AP(tensor=a.tensor, offset=a.offset + part0 * w + col0,
                   ap=[[w, nparts]] + dims)


def dram_ap(t, off, dims):
    a = t if isinstance(t, bass.AP) else t[:]
    return bass.AP(tensor=a.tensor, offset=a.offset + off, ap=dims)


def build_nc():
    nc = bacc.Bacc("TRN2", num_devices=N_CORES, debug=False)

    xs = nc.dram_tensor("xs", (NCH, XSLOTS, 192), F32, kind="ExternalInput").ap()
    ym = nc.dram_tensor("ym", (2, 1), F32, kind="ExternalInput").ap()
    bmask = nc.dram_tensor("bmask", (1, 36), F32, kind="ExternalInput").ap()
    params = {}
    for li in (1, 2):
        for name, shp in PARAM_SHAPES.items():
            params[(li, name)] = nc.dram_tensor(
                f"{name}_{li}", shp, F32, kind="ExternalInput").ap()
    out_d = nc.dram_tensor("o", (NCH, 96, 192), F32, kind="ExternalOutput").ap()

    with tile.TileContext(nc) as tc:
        with (
            tc.tile_pool(name="persist", bufs=1) as per,
            tc.tile_pool(name="stag", bufs=2) as stag,
            tc.tile_pool(name="gat", bufs=1) as gat,
            tc.tile_pool(name="pconv", bufs=5, space="PSUM") as pconv,
            tc.tile_pool(name="pgat", bufs=3, space="PSUM") as pgat,
            tc.tile_pool(name="dram", bufs=2, space="DRAM") as dpool,
        ):
            xx = per.tile([128, XW], F16)
            yy = per.tile([128, YW], F16)
            outt = per.tile([NCH, OUTW], F32)
            ident = per.tile([NCH, NCH], F32)
            make_identity(nc, ident[:])
            epst = per.tile([NCH, 1], F32)
            nc.vector.memset(epst[:], EPS)
            ymt = per.tile([128, 2], F32)
            nc.sync.dma_start(out=ymt[:, 0:1], in_=dram_ap(ym, 0, [[0, 128], [1, 1]]))
            nc.sync.dma_start(out=ymt[:, 1:2], in_=dram_ap(ym, 1, [[0, 128], [1, 1]]))
            bmaskt = per.tile([NCH, 36], F32)
            nc.sync.dma_start(out=bmaskt[:], in_=dram_ap(bmask, 0, [[0, NCH], [1, 36]]))

            # param tiles
            pt = {}
            for li in (1, 2):
                for name, shp in PARAM_SHAPES.items():
                    t = per.tile(list(shp), F32, tag=f"{name}_{li}")
                    nc.sync.dma_start(out=t[:], in_=params[(li, name)])
                    pt[(li, name)] = t

            # ---- zero pad structure of xx / yy ----
            for t, nslots, w in ((xx, XSLOTS, XW), (yy, YSLOTS, YW)):
                nc.gpsimd.memset(ap_of(t, 0, 128, 0, [[1, XOFF + 1]]), 0.0)
                nc.gpsimd.memset(
                    ap_of(t, 0, 128, XOFF + 193, [[PW, nslots - 1], [1, 2]]), 0.0)
                nc.gpsimd.memset(
                    ap_of(t, 0, 128, XOFF + (nslots - 1) * PW + 193, [[1, 1]]), 0.0)
            # yy data region starts zeroed (conv1 skips boundary slots 0/99)
            nc.gpsimd.memset(ap_of(yy, 0, 128, XOFF, [[PW, 1], [1, PW]]), 0.0)
            nc.gpsimd.memset(
                ap_of(yy, 0, 128, XOFF + 99 * PW, [[1, YW - XOFF - 99 * PW]]), 0.0)
            # upper-half cols of yy below first written block
            nc.gpsimd.memset(ap_of(yy, 64, 64, XOFF + 1 * PW, [[1, PW]]), 0.0)

            # ---- load x: DMA chunks -> fp32 staging -> fp16 xx ----
            CH = 8
            chunks = [(t0, min(CH, XSLOTS - t0)) for t0 in range(0, XSLOTS, CH)]
            for i, (t0, nt) in enumerate(chunks):
                st = stag.tile([NCH, CH * 192], F32, tag="xstag")
                nc.sync.dma_start(out=st[0:NCH, 0:nt * 192], in_=xs[:, t0:t0 + nt, :])
                eng = nc.scalar if i % 2 == 0 else nc.vector
                src = ap_of(st, 0, NCH, 0, [[192, nt], [1, 192]])
                dst = ap_of(xx, 0, NCH, XOFF + t0 * PW + 1, [[PW, nt], [1, 192]])
                if i % 2 == 0:
                    nc.scalar.copy(out=dst, in_=src)
                else:
                    nc.vector.tensor_copy(out=dst, in_=src)
                # upper half = fp16 copy shifted one slot down (partition cross -> DMA)
                ut0, unt = (t0, nt) if t0 > 0 else (1, nt - 1)
                nc.sync.dma_start(
                    out=ap_of(xx, 64, 64, XOFF + (ut0 - 1) * PW + 1, [[PW, unt], [1, 192]]),
                    in_=ap_of(xx, 0, 64, XOFF + ut0 * PW + 1, [[PW, unt], [1, 192]]),
                )

            # ---- helpers ----
            def gl_reduce(src, slot_start, layer):
                """3 sub-band reduces (pair trick over both halves) -> (64,9) sbuf."""
                glp = gat.tile([128, 9], F32, tag="glp")
                for sub in range(3):
                    s0 = slot_start + sub * 32
                    inap = ap_of(src, 0, 128, XOFF + s0 * PW + 1,
                                 [[64, 3], [2 * PW, 16], [1, 64]])
                    nc.vector.tensor_reduce(
                        out=glp[0:128, sub * 3:sub * 3 + 3], in_=inap,
                        axis=mybir.AxisListType.XY, op=ALU.add)
                glpu = gat.tile([NCH, 9], F32, tag="glpu")
                nc.sync.dma_start(out=glpu[:], in_=glp[64:128, :])
                part = gat.tile([NCH, 9], F32, tag="glpart")
                nc.vector.tensor_add(out=part[:], in0=glp[0:64, :], in1=glpu[:])
                return part

            def all_gather_gl(part):
                bin_ = dpool.tile([NCH, 9], F32, tag="agin")
                agout = dpool.tile([N_CORES * NCH, 9], F32, tag="agout")
                nc.gpsimd.dma_start(out=bin_[:], in_=part[:])
                nc.gpsimd.collective_compute(
                    "AllGather", ALU.bypass,
                    replica_groups=[list(range(N_CORES))],
                    ins=[bin_.opt()], outs=[agout.opt()])
                GG = gat.tile([NCH, 72], F32, tag="GG")
                nc.sync.dma_start(
                    out=GG[:],
                    in_=dram_ap(agout, 0, [[9, NCH], [576, 8], [1, 9]]))
                glx = gat.tile([NCH, 36], F32, tag="glx")
                nc.vector.tensor_tensor(
                    out=glx[:],
                    in0=ap_of(GG, 0, NCH, 0, [[18, 4], [6, 3], [1, 3]]),
                    in1=ap_of(GG, 0, NCH, 3, [[18, 4], [6, 3], [1, 3]]),
                    op=ALU.add)
                nc.vector.tensor_scalar_mul(out=glx[:], in0=glx[:],
                                            scalar1=1.0 / 4096.0)
                gbounce = dpool.tile([9, 256], F32, tag="glxb")
                # write k-major: dram flat(k, b, c) = k*256 + b*64 + c
                for b in range(4):
                    nc.sync.dma_start(
                        out=dram_ap(gbounce, 64 * b, [[1, NCH], [256, 9]]),
                        in_=glx[0:NCH, 9 * b:9 * b + 9])
                glT = gat.tile([9, 256], F32, tag="glT")
                nc.sync.dma_start(out=glT[:], in_=gbounce[:])
                return glT

            def transpose4(src, tags):
                """4x PE-transpose (64,5) batch slices of src(64,20) -> sbuf (5,256)."""
                ps = pgat.tile([5, 256], F32, tag="pg")
                for b in range(4):
                    nc.tensor.transpose(
                        out=ps[0:5, 64 * b:64 * b + 64],
                        in_=src[0:NCH, 5 * b:5 * b + 5], identity=ident[:])
                sb = gat.tile([5, 256], F32, tag=tags)
                nc.vector.tensor_copy(out=sb[:], in_=ps[:])
                return sb

            def gating(li, glT):
                """Full gating math for layer li; returns (LP fp16 (128,192),
                LS fp16 (64,192)) conv lhsT tiles."""
                P = lambda n: pt[(li, n)]
                # out = gl @ ce.T   (64c, 4b*5)
                ops_ = pgat.tile([NCH, 20], F32, tag="pg")
                for b in range(4):
                    nc.tensor.matmul(ops_[:, 5 * b:5 * b + 5],
                                     glT[0:9, 64 * b:64 * b + 64], P("ceT")[:],
                                     start=True, stop=True)
                O = gat.tile([NCH, 20], F32, tag="O")
                nc.scalar.copy(out=O[:], in_=ops_[:])
                # w1/w2 = aff @ out
                wps = pgat.tile([NCH, 40], F32, tag="pg")
                nc.tensor.matmul(wps[:, 0:20], P("aff1T")[:], O[:], start=True, stop=True)
                nc.tensor.matmul(wps[:, 20:40], P("aff2T")[:], O[:], start=True, stop=True)
                W12 = gat.tile([NCH, 40], F32, tag="W12")
                nc.scalar.copy(out=W12[:], in_=wps[:])
                # affinityT_b = w2_b.T @ w1_b  (5,5) blocks
                atps = pgat.tile([5, 20], F32, tag="pg")
                for b in range(4):
                    nc.tensor.matmul(atps[0:5, 5 * b:5 * b + 5],
                                     W12[0:NCH, 20 + 5 * b:25 + 5 * b],
                                     W12[0:NCH, 5 * b:5 * b + 5],
                                     start=True, stop=True)
                ATs = gat.tile([5, 20], F32, tag="ATs")
                nc.scalar.copy(out=ATs[:], in_=atps[:])
                OTs = transpose4(O, "OTs")
                # out1_b (5i, 64c)
                o1ps = pgat.tile([5, 256], F32, tag="pg")
                for b in range(4):
                    nc.tensor.matmul(o1ps[0:5, 64 * b:64 * b + 64],
                                     ATs[0:5, 5 * b:5 * b + 5],
                                     OTs[0:5, 64 * b:64 * b + 64],
                                     start=True, stop=True)
                O1 = gat.tile([5, 256], F32, tag="O1")
                nc.scalar.copy(out=O1[:], in_=o1ps[:])
                # WW = wg @ wr
                wwps = pgat.tile([NCH, NCH], F32, tag="pg")
                nc.tensor.matmul(wwps[:], P("wgT")[:], P("wr")[:], start=True, stop=True)
                WW = gat.tile([NCH, NCH], F32, tag="WW")
                nc.scalar.copy(out=WW[:], in_=wwps[:])
                # out1T via transposes (5,64)->(64,5)
                o1tps = pgat.tile([NCH, 20], F32, tag="pg")
                for b in range(4):
                    nc.tensor.transpose(
                        out=o1tps[0:NCH, 5 * b:5 * b + 5],
                        in_=O1[0:5, 64 * b:64 * b + 64], identity=ident[0:5, 0:5])
                O1T = gat.tile([NCH, 20], F32, tag="O1T")
                nc.scalar.copy(out=O1T[:], in_=o1tps[:])
                # out3T_b = WW.T-> lhsT=WW @ out1T_b
                o3ps = pgat.tile([NCH, 20], F32, tag="pg")
                for b in range(4):
                    nc.tensor.matmul(o3ps[:, 5 * b:5 * b + 5], WW[:],
                                     O1T[0:NCH, 5 * b:5 * b + 5],
                                     start=True, stop=True)
                # ce2 = (out3T + 1) * out
                ce2 = gat.tile([NCH, 20], F32, tag="ce2")
                nc.vector.scalar_tensor_tensor(
                    out=ce2[:], in0=o3ps[:], scalar=1.0, in1=O[:],
                    op0=ALU.add, op1=ALU.mult)
                # BN stats over 20 cols
                stats = gat.tile([NCH, nc.vector.BN_STATS_DIM], F32, tag="bnst")
                nc.vector.bn_stats(out=stats[:], in_=ce2[:])
                mv = gat.tile([NCH, nc.vector.BN_AGGR_DIM], F32, tag="bnmv")
                nc.vector.bn_aggr(out=mv[:], in_=stats[:])
                rstd = gat.tile([NCH, 1], F32, tag="rstd")
                nc.scalar.activation(out=rstd[:], in_=mv[:, 1:2], func=AF.Sqrt,
                                     bias=epst[:], scale=1.0)
                nc.vector.reciprocal(out=rstd[:], in_=rstd[:])
                # normalized (shared by t and u paths)
                bn1 = gat.tile([NCH, 20], F32, tag="bn1")
                nc.vector.tensor_scalar(
                    out=bn1[:], in0=ce2[:], scalar1=mv[:, 0:1], scalar2=rstd[:],
                    op0=ALU.subtract, op1=ALU.mult)
                # t path: relu(bn1*g+b) @ gd.T
                tr = gat.tile([NCH, 20], F32, tag="tr")
                nc.vector.tensor_scalar(
                    out=tr[:], in0=bn1[:], scalar1=P("bn")[:, 0:1],
                    scalar2=P("bn")[:, 1:2], op0=ALU.mult, op1=ALU.add)
                nc.vector.tensor_scalar_max(out=tr[:], in0=tr[:], scalar1=0.0)
                tTs = transpose4(tr, "tTs")
                g1ps = pgat.tile([NCH, 36], F32, tag="pg")
                for b in range(4):
                    nc.tensor.matmul(g1ps[:, 9 * b:9 * b + 9],
                                     tTs[0:5, 64 * b:64 * b + 64], P("gdT")[:],
                                     start=True, stop=True)
                G1 = gat.tile([NCH, 36], F32, tag="G1")
                nc.scalar.copy(out=G1[:], in_=g1ps[:])
                # u path
                ur = gat.tile([NCH, 20], F32, tag="ur")
                nc.vector.tensor_scalar(
                    out=ur[:], in0=bn1[:], scalar1=P("bn")[:, 2:3],
                    scalar2=P("bn")[:, 3:4], op0=ALU.mult, op1=ALU.add)
                nc.vector.tensor_scalar_max(out=ur[:], in0=ur[:], scalar1=0.0)
                urb = dpool.tile([NCH, 20], F32, tag="urb")
                nc.sync.dma_start(out=urb[:], in_=ur[:])
                U = gat.tile([16, 80], F32, tag="U")
                nc.sync.dma_start(
                    out=U[:], in_=dram_ap(urb, 0, [[20, 16], [320, 4], [1, 20]]))
                u2ps = pgat.tile([16, 80], F32, tag="pg")
                nc.tensor.matmul(u2ps[:], P("ciT")[:], U[:], start=True, stop=True)
                U2 = gat.tile([16, 80], F32, tag="U2")
                nc.scalar.copy(out=U2[:], in_=u2ps[:])
                u2b = dpool.tile([NCH, 20], F32, tag="u2b")
                nc.sync.dma_start(
                    out=dram_ap(u2b, 0, [[20, 16], [320, 4], [1, 20]]), in_=U2[:])
                u3 = gat.tile([NCH, 20], F32, tag="u3")
                nc.sync.dma_start(out=u3[:], in_=u2b[:])
                # BN3 + relu
                nc.vector.bn_stats(out=stats[:], in_=u3[:])
                nc.vector.bn_aggr(out=mv[:], in_=stats[:])
                rstd3 = gat.tile([NCH, 1], F32, tag="rstd3")
                nc.scalar.activation(out=rstd3[:], in_=mv[:, 1:2], func=AF.Sqrt,
                                     bias=epst[:], scale=1.0)
                nc.vector.reciprocal(out=rstd3[:], in_=rstd3[:])
                uu = gat.tile([NCH, 20], F32, tag="uu")
                nc.vector.tensor_scalar(
                    out=uu[:], in0=u3[:], scalar1=mv[:, 0:1], scalar2=rstd3[:],
                    op0=ALU.subtract, op1=ALU.mult)
                nc.vector.tensor_scalar(
                    out=uu[:], in0=uu[:], scalar1=P("bn")[:, 4:5],
                    scalar2=P("bn")[:, 5:6], op0=ALU.mult, op1=ALU.add)
                nc.vector.tensor_scalar_max(out=uu[:], in0=uu[:], scalar1=0.0)
                uTs = transpose4(uu, "uTs")
                g2ps = pgat.tile([NCH, 36], F32, tag="pg")
                for b in range(4):
                    nc.tensor.matmul(g2ps[:, 9 * b:9 * b + 9],
                                     uTs[0:5, 64 * b:64 * b + 64], P("gd2T")[:],
                                     start=True, stop=True)
                G2 = gat.tile([NCH, 36], F32, tag="G2")
                nc.scalar.copy(out=G2[:], in_=g2ps[:])
                # select this core's batch: masked reduce over b
                g1sel = gat.tile([NCH, 9], F32, tag="g1sel")
                g2sel = gat.tile([NCH, 9], F32, tag="g2sel")
                tmp = gat.tile([NCH, 36], F32, tag="gselt")
                for gsel, Gt in ((g1sel, G1), (g2sel, G2)):
                    nc.vector.tensor_mul(out=tmp[:], in0=Gt[:], in1=bmaskt[:])
                    nc.vector.tensor_reduce(
                        out=gsel[:],
                        in_=ap_of(tmp, 0, NCH, 0, [[1, 9], [9, 4]]),
                        axis=mybir.AxisListType.X, op=ALU.add)
                # broadcast g1sel over oc partitions
                g1b_d = dpool.tile([NCH, 9], F32, tag="g1bd")
                nc.sync.dma_start(out=g1b_d[:], in_=g1sel[:])
                g1b = gat.tile([NCH, 576], F32, tag="g1b")
                nc.sync.dma_start(
                    out=g1b[:], in_=dram_ap(g1b_d, 0, [[0, NCH], [9, NCH], [1, 9]]))
                # kern = sigmoid(g1b + g2sel) * wconv   (64oc, c*9+k)
                ksb = gat.tile([NCH, 576], F32, tag="ksb")
                nc.vector.tensor_tensor(
                    out=ksb[:], in0=g1b[:],
                    in1=ap_of(g2sel, 0, NCH, 0, [[0, NCH], [1, 9]]), op=ALU.add)
                nc.scalar.activation(out=ksb[:], in_=ksb[:], func=AF.Sigmoid)
                nc.vector.tensor_mul(out=ksb[:], in0=ksb[:], in1=P("wconv")[:])
                # pair-adjacent k-blocks: cols (p, h, c) k=p+3h, then singles
                kK = gat.tile([NCH, 576], F32, tag="kK")
                nc.vector.tensor_copy(
                    out=kK[0:NCH, 0:384],
                    in_=ap_of(ksb, 0, NCH, 0, [[1, 3], [3, 2], [9, NCH]]))
                nc.vector.tensor_copy(
                    out=kK[0:NCH, 384:576],
                    in_=ap_of(ksb, 0, NCH, 6, [[1, 3], [9, NCH]]))
                # transposes -> lhsT tiles
                LP = per.tile([128, 192], F16, tag=f"LP{li}")
                LS = per.tile([NCH, 192], F16, tag=f"LS{li}")
                for p in range(3):
                    tps = pgat.tile([128, NCH], F32, tag="pg")
                    nc.tensor.transpose(
                        out=tps[:], in_=kK[0:NCH, 128 * p:128 * p + 128],
                        identity=ident[:])
                    nc.vector.tensor_copy(out=LP[:, 64 * p:64 * p + 64], in_=tps[:])
                for sq in range(3):
                    tss = pgat.tile([NCH, NCH], F32, tag="pg")
                    nc.tensor.transpose(
                        out=tss[:], in_=kK[0:NCH, 384 + 64 * sq:448 + 64 * sq],
                        identity=ident[:])
                    nc.vector.tensor_copy(out=LS[:, 64 * sq:64 * sq + 64], in_=tss[:])
                return LP, LS

            def conv_layer(src, LP, LS, lo, hi, shift, epilogue):
                nblk = 0
                for c0 in range(lo, hi, CONV_BLK):
                    bw = min(CONV_BLK, hi - c0)
                    ps = pconv.tile([NCH, CONV_BLK], F32, tag="convps")
                    base = XOFF + c0 + shift
                    for i, da in enumerate(PAIR_DELTAS):
                        nc.tensor.matmul(
                            ps[:, 0:bw], LP[:, 64 * i:64 * i + 64],
                            ap_of(src, 0, 128, base + da, [[1, bw]]),
                            start=(i == 0), stop=False)
                    for j, d in enumerate(SINGLE_DELTAS):
                        nc.tensor.matmul(
                            ps[:, 0:bw], LS[0:NCH, 64 * j:64 * j + 64],
                            ap_of(src, 0, NCH, base + d, [[1, bw]]),
                            start=False, stop=(j == 2))
                    epilogue(c0, bw, ps, nblk)
                    nblk += 1

            # ================= layer 1 =================
            glT1 = all_gather_gl(gl_reduce(xx, 3, 1))
            LP1, LS1 = gating(1, glT1)

            def epi1(c0, bw, ps, nblk):
                dst = ap_of(yy, 0, NCH, XOFF + c0, [[1, bw]])
                if nblk % 2 == 0:
                    nc.scalar.activation(out=dst, in_=ps[0:NCH, 0:bw], func=AF.Relu)
                else:
                    nc.vector.tensor_scalar_max(out=dst, in0=ps[0:NCH, 0:bw],
                                                scalar1=0.0)
                nc.sync.dma_start(
                    out=ap_of(yy, 64, 64, XOFF + c0 - 194, [[1, bw]]),
                    in_=ap_of(yy, 0, 64, XOFF + c0, [[1, bw]]))

            conv_layer(xx, LP1, LS1, L1_LO, L1_HI, 194, epi1)

            # pad-fix: re-zero pad cols (both halves), then boundary masks
            nc.gpsimd.memset(
                ap_of(yy, 0, 128, XOFF + 193, [[PW, YSLOTS - 1], [1, 2]]), 0.0)
            for part0 in (0, 64):
                off = part0 and -194 or 0
                nc.vector.tensor_scalar_mul(
                    out=ap_of(yy, part0, 64, XOFF + PW + off, [[1, PW]]),
                    in0=ap_of(yy, part0, 64, XOFF + PW + off, [[1, PW]]),
                    scalar1=ymt[part0:part0 + 64, 0:1])
                nc.vector.tensor_scalar_mul(
                    out=ap_of(yy, part0, 64, XOFF + 98 * PW + off, [[1, PW]]),
                    in0=ap_of(yy, part0, 64, XOFF + 98 * PW + off, [[1, PW]]),
                    scalar1=ymt[part0:part0 + 64, 1:2])

            # ================= layer 2 =================
            glT2 = all_gather_gl(gl_reduce(yy, 2, 2))
            LP2, LS2 = gating(2, glT2)

            flush_state = {"rows": 0}

            def epi2(c0, bw, ps, nblk):
                nc.vector.tensor_tensor(
                    out=outt[0:NCH, c0 - L2_LO:c0 - L2_LO + bw],
                    in0=ps[0:NCH, 0:bw],
                    in1=ap_of(xx, 0, NCH, XOFF + c0 + 194, [[1, bw]]),
                    op=ALU.add)
                done_rows = (c0 + bw - L2_LO) // PW
                if done_rows - flush_state["rows"] >= 12 or (c0 + bw) == L2_HI:
                    r0, r1 = flush_state["rows"], done_rows
                    if (c0 + bw) == L2_HI:
                        r1 = 96
                    if r1 > r0:
                        nc.sync.dma_start(
                            out=out_d[:, r0:r1, :],
                            in_=ap_of(outt, 0, NCH, r0 * PW + 1,
                                      [[PW, r1 - r0], [1, 192]]))
                    flush_state["rows"] = r1

            conv_layer(yy, LP2, LS2, L2_LO, L2_HI, 0, epi2)

    nc.compile()
    return nc


def host_prep(x, p1, p2):
    """Build per-core input maps from full inputs."""
    x = np.ascontiguousarray(np.asarray(x, dtype=np.float32))
    in_maps = []
    for core in range(N_CORES):
        b, h = core // 2, core % 2
        r0 = 96 * h
        sh = np.zeros((NCH, XSLOTS, 192), np.float32)
        lo, hi = r0 - 3, r0 + 99
        slo, shi = max(lo, 0), min(hi, 192)
        sh[:, slo - lo:shi - lo, :] = x[b, :, slo:shi, :]
        m = {"xs": sh,
             "ym": np.array([[0.0], [1.0]] if h == 0 else [[1.0], [0.0]], np.float32),
             "bmask": np.kron(np.eye(4, dtype=np.float32)[b], np.ones(9, np.float32)
                              ).reshape(1, 36)}
        for li, p in ((1, p1), (2, p2)):
            p = {k: np.asarray(v, dtype=np.float32) for k, v in p.items()}
            m[f"ceT_{li}"] = np.ascontiguousarray(p["ce"].T)
            m[f"aff1T_{li}"] = np.ascontiguousarray(p["aff1"].T)
            m[f"aff2T_{li}"] = np.ascontiguousarray(p["aff2"].T)
            m[f"wgT_{li}"] = np.ascontiguousarray(p["wg"].T)
            m[f"wr_{li}"] = np.ascontiguousarray(p["wr"])
            m[f"gdT_{li}"] = np.ascontiguousarray(p["gd"].T)
            m[f"gd2T_{li}"] = np.ascontiguousarray(p["gd2"].T)
            m[f"ciT_{li}"] = np.ascontiguousarray(p["ci"].T)
            m[f"wconv_{li}"] = np.ascontiguousarray(p["wconv"].reshape(NCH, 576))
            m[f"bn_{li}"] = np.ascontiguousarray(np.stack(
                [p["ce_bn_g"], p["ce_bn_b"], p["ci_bn2_g"], p["ci_bn2_b"],
                 p["ci_bn_g"], p["ci_bn_b"]], axis=1))
        in_maps.append(m)
    return in_maps


_CACHE = {}


def kernel(x, p1, p2):
    import concourse.bass_utils as bass_utils
    in_maps = host_prep(x, p1, p2)
    if "nc" not in _CACHE:
        _CACHE["nc"] = build_nc()
    nc = _CACHE["nc"]
    res = bass_utils.run_bass_kernel_spmd(
        nc, in_maps, core_ids=list(range(N_CORES)), trace=False)
    out = np.empty((4, NCH, 192, 192), np.float32)
    for core in range(N_CORES):
        b, h = core // 2, core % 2
        out[b, :, 96 * h:96 * h + 96, :] = res.results[core]["o"]
    return out


# revision 19
# speedup vs baseline: 1.0927x; 1.0927x over previous
"""Trainium2 Bass kernel for nn_CRABLayer (dynamic gated 3x3 conv x2 + residual).

Sharding: 8 cores = (batch b in 0..3) x (image half h in 0..1); each core
computes its (b, 96-row) output slab. The tiny cross-batch gating statistics
(adaptive-avg-pool "gl") are combined with one AllGather per layer.

Layout: per-channel padded rows of width 194 (1 zero col each side), flattened
so a 3x3 conv becomes 9 column-shifted matmuls; shifts differing by 194 are
K-stacked (x in SBUF partitions 0:64, x<<194 in 64:128) into K=128 matmuls.
Conv matmuls run in fp16 (fp32 PSUM accumulation) with 2x PE column tiling
(two 512-px blocks concurrently on array column halves); gating math is fp32.
"""
import numpy as np
import concourse.bass as bass
import concourse.bacc as bacc
import concourse.tile as tile
import concourse.mybir as mybir
from concourse.masks import make_identity

F32 = mybir.dt.float32
F16 = mybir.dt.float16
AF = mybir.ActivationFunctionType
ALU = mybir.AluOpType

NCH = 64
EPS = 1e-5
PW = 194
XSLOTS, YSLOTS = 102, 100
XOFF = 1
XW = XOFF + XSLOTS * PW          # 19789
YW = XOFF + YSLOTS * PW          # 19401
OUTW = 96 * PW                   # 18624
PAIR_DELTAS = [-195, -194, -193]
SINGLE_DELTAS = [193, 194, 195]
KPERM = [0, 3, 1, 4, 2, 5, 6, 7, 8]   # pair-adjacent k order
L1_LO, L1_HI = 195, 19205        # y1 slots [1, 99), excl boundary pads
L2_LO, L2_HI = 388, 19012        # out slots [2, 98)
N_CORES = 8
CONV_BLK = 512

# packed param layout: (64, PCOLS) fp32; col ranges per param
_PACK = [("aff1T", (NCH, NCH)), ("aff2T", (NCH, NCH)), ("wgT", (NCH, NCH)),
         ("wr", (NCH, NCH)), ("wconvP", (NCH, 576)), ("bn", (NCH, 6)),
         ("ceBD", (36, 20)), ("gdBD", (20, 36)), ("gd2BD", (20, 36)),
         ("ciBD", (NCH, NCH)), ("atmask", (20, 20))]
PCOLS = sum(s[1] for _, s in _PACK)
POFF = {}
_c = 0
for _n, _s in _PACK:
    POFF[_n] = (_c, _s)
    _c += _s[1]


def ap_of(t, part0, nparts, col0, dims):
    """Custom AP into a pool tile t: partitions [part0, part0+nparts),
    free pattern dims=[[step, count], ...] starting at column col0."""
    a = t[:]
    w = a.ap[0][0]
    lo = hi = 0
    for s, c in dims:
        if s >= 0:
            hi += s * (c - 1)
        else:
            lo += s * (c - 1)
    assert col0 + lo >= 0 and col0 + hi <= w - 1, (col0, dims, w)
    assert 0 <= part0 and part0 + nparts <= a.ap[0][1], (part0, nparts)
    return bass.AP(tensor=a.tensor, offset=a.offset + part0 * w + col0,
                   ap=[[w, nparts]] + dims)


def dram_ap(t, off, dims):
    a = t if isinstance(t, bass.AP) else t[:]
    return bass.AP(tensor=a.tensor, offset=a.offset + off, ap=dims)


def build_nc(loop=0, debug=False):
    nc = bacc.Bacc("TRN2", num_devices=N_CORES, debug=False)

    xs = nc.dram_tensor("xs", (NCH, XSLOTS * PW), F16, kind="ExternalInput").ap()
    ym = nc.dram_tensor("ym", (2, 1), F32, kind="ExternalInput").ap()
    bmask = nc.dram_tensor("bmask", (1, 36), F32, kind="ExternalInput").ap()
    pk = {li: nc.dram_tensor(f"pk_{li}", (NCH, PCOLS), F32,
                             kind="ExternalInput").ap() for li in (1, 2)}
    out_d = nc.dram_tensor("o", (NCH, 96, 192), F32, kind="ExternalOutput").ap()
    dbg_d = (nc.dram_tensor("dbg", (128, YW), F16, kind="ExternalOutput").ap()
             if debug else None)

    with tile.TileContext(nc) as tc:
        with (
            tc.tile_pool(name="persist", bufs=1) as per,
            tc.tile_pool(name="gat", bufs=1) as gat,
            tc.tile_pool(name="pconv", bufs=5, space="PSUM") as pconv,
            tc.tile_pool(name="pgat", bufs=3, space="PSUM") as pgat,
            tc.tile_pool(name="dram", bufs=2, space="DRAM") as dpool,
        ):
            xx = per.tile([128, XW], F16)
            yy = per.tile([128, YW], F16)
            outt = per.tile([NCH, OUTW], F32)
            ident = per.tile([NCH, NCH], F32)
            make_identity(nc, ident[:])
            epst = per.tile([NCH, 1], F32)
            nc.vector.memset(epst[:], EPS)
            ymt = per.tile([128, 2], F32)
            bmaskt = per.tile([NCH, 36], F32)
            pk1t = per.tile([NCH, PCOLS], F32, tag="pk1")
            pk2t = per.tile([NCH, PCOLS], F32, tag="pk2")
            pkt = {1: pk1t, 2: pk2t}

            def P(li, name):
                c0, (p, w) = POFF[name]
                return pkt[li][0:p, c0:c0 + w]

            def body():
                nc.sync.dma_start(out=ymt[:, 0:1],
                                  in_=dram_ap(ym, 0, [[0, 128], [1, 1]]))
                nc.sync.dma_start(out=ymt[:, 1:2],
                                  in_=dram_ap(ym, 1, [[0, 128], [1, 1]]))
                nc.sync.dma_start(out=bmaskt[:],
                                  in_=dram_ap(bmask, 0, [[0, NCH], [1, 36]]))
                for li in (1, 2):
                    nc.scalar.dma_start(out=pkt[li][:], in_=pk[li])

                # ---- zero pad structure (xx data+pads come zeroed from host) ----
                nc.gpsimd.memset(ap_of(xx, 0, 128, 0, [[1, XOFF]]), 0.0)
                nc.gpsimd.memset(
                    ap_of(xx, 64, 64, XOFF + 101 * PW, [[1, PW]]), 0.0)
                nc.gpsimd.memset(ap_of(yy, 0, 128, 0, [[1, XOFF + 1]]), 0.0)
                nc.gpsimd.memset(
                    ap_of(yy, 0, 128, XOFF + 193, [[PW, YSLOTS - 1], [1, 2]]), 0.0)
                nc.gpsimd.memset(
                    ap_of(yy, 0, 128, XOFF + (YSLOTS - 1) * PW + 193, [[1, 1]]), 0.0)
                # yy boundary slots 0/99 stay zero (conv1 writes slots 1..98)
                nc.gpsimd.memset(ap_of(yy, 0, 128, XOFF, [[PW, 1], [1, PW]]), 0.0)
                nc.gpsimd.memset(
                    ap_of(yy, 0, 128, XOFF + 99 * PW, [[1, YW - XOFF - 99 * PW]]), 0.0)

                # ---- load x (fp16, host-prepadded): fully contiguous DMAs ----
                bounds = [0, 26, 52, 77, XSLOTS]
                for ci in range(4):
                    t0, t1 = bounds[ci], bounds[ci + 1]
                    hs = t1 - t0
                    eng = nc.sync if ci % 2 == 0 else nc.scalar
                    eng.dma_start(
                        out=ap_of(xx, 0, NCH, XOFF + t0 * PW, [[1, hs * PW]]),
                        in_=dram_ap(xs, t0 * PW, [[XSLOTS * PW, NCH], [1, hs * PW]]))
                    ut0 = max(t0 * PW, PW)
                    un = t1 * PW - ut0
                    eng.dma_start(
                        out=ap_of(xx, 64, 64, XOFF + ut0 - PW, [[1, un]]),
                        in_=ap_of(xx, 0, 64, XOFF + ut0, [[1, un]]))

                # ---- helpers ----
                def gl_reduce(src, slot_start):
                    glp = gat.tile([128, 9], F32, tag="glp")
                    for sub in range(3):
                        s0 = slot_start + sub * 32
                        inap = ap_of(src, 0, 128, XOFF + s0 * PW + 1,
                                     [[64, 3], [2 * PW, 16], [1, 64]])
                        nc.vector.tensor_reduce(
                            out=glp[0:128, sub * 3:sub * 3 + 3], in_=inap,
                            axis=mybir.AxisListType.XY, op=ALU.add)
                    glpu = gat.tile([NCH, 9], F32, tag="glpu")
                    nc.sync.dma_start(out=glpu[:], in_=glp[64:128, :])
                    part = gat.tile([NCH, 9], F32, tag="glpart")
                    nc.vector.tensor_add(out=part[:], in0=glp[0:64, :], in1=glpu[:])
                    return part

                def all_gather_gl(part):
                    bin_ = dpool.tile([NCH, 9], F32, tag="agin")
                    agout = dpool.tile([N_CORES * NCH, 9], F32, tag="agout")
                    nc.gpsimd.dma_start(out=bin_[:], in_=part[:])
                    nc.gpsimd.collective_compute(
                        "AllGather", ALU.bypass,
                        replica_groups=[list(range(N_CORES))],
                        ins=[bin_.opt()], outs=[agout.opt()])
                    GG = gat.tile([NCH, 72], F32, tag="GG")
                    nc.sync.dma_start(
                        out=GG[:], in_=dram_ap(agout, 0, [[9, NCH], [576, 8], [1, 9]]))
                    glx = gat.tile([NCH, 36], F32, tag="glx")
                    nc.vector.tensor_tensor(
                        out=glx[:],
                        in0=ap_of(GG, 0, NCH, 0, [[18, 4], [6, 3], [1, 3]]),
                        in1=ap_of(GG, 0, NCH, 3, [[18, 4], [6, 3], [1, 3]]),
                        op=ALU.add)
                    nc.vector.tensor_scalar_mul(out=glx[:], in0=glx[:],
                                                scalar1=1.0 / 4096.0)
                    gbounce = dpool.tile([36, NCH], F32, tag="glxb")
                    # dram flat = (b*9+k)*64 + c
                    nc.sync.dma_start(
                        out=dram_ap(gbounce, 0, [[1, NCH], [576, 4], [64, 9]]),
                        in_=ap_of(glx, 0, NCH, 0, [[9, 4], [1, 9]]))
                    glT = gat.tile([36, NCH], F32, tag="glT")
                    nc.sync.dma_start(out=glT[:], in_=gbounce[:])
                    return glT

                def trans(src, np_, nf, tagp):
                    """PE transpose (np_, nf) -> psum (nf, np_) -> sbuf."""
                    ps = pgat.tile([nf if nf > 20 else 20, 64], F32, tag="pg")
                    nc.tensor.transpose(out=ps[0:nf, 0:np_], in_=src,
                                        identity=ident[0:np_, 0:np_])
                    sb = gat.tile([nf if nf > 20 else 20, 64], F32, tag=tagp)
                    nc.vector.tensor_copy(out=sb[0:nf, 0:np_], in_=ps[0:nf, 0:np_])
                    return sb

                def gating(li, glT):
                    # out = gl @ ce.T for all batches: one block-diag matmul
                    ops_ = pgat.tile([NCH, 20], F32, tag="pg")
                    nc.tensor.matmul(ops_[:], glT[:], P(li, "ceBD"),
                                     start=True, stop=True)
                    O = gat.tile([NCH, 20], F32, tag="O")
                    nc.vector.tensor_copy(out=O[:], in_=ops_[:])
                    wps = pgat.tile([NCH, 40], F32, tag="pg")
                    nc.tensor.matmul(wps[:, 0:20], P(li, "aff1T"), O[:],
                                     start=True, stop=True)
                    nc.tensor.matmul(wps[:, 20:40], P(li, "aff2T"), O[:],
                                     start=True, stop=True)
                    W12 = gat.tile([NCH, 40], F32, tag="W12")
                    nc.vector.tensor_copy(out=W12[:], in_=wps[:])
                    # affinityT blocks = w2_b.T @ w1_b; mask off cross-batch
                    atps = pgat.tile([20, 20], F32, tag="pg")
                    nc.tensor.matmul(atps[:], W12[0:NCH, 20:40], W12[0:NCH, 0:20],
                                     start=True, stop=True)
                    ATm = gat.tile([20, 20], F32, tag="ATm")
                    nc.vector.tensor_mul(out=ATm[:], in0=atps[:], in1=P(li, "atmask"))
                    OTs = trans(O[:], NCH, 20, "OTs")
                    o1ps = pgat.tile([20, 64], F32, tag="pg")
                    nc.tensor.matmul(o1ps[:], ATm[:], OTs[0:20, 0:NCH],
                                     start=True, stop=True)
                    O1 = gat.tile([20, 64], F32, tag="O1")
                    nc.vector.tensor_copy(out=O1[:], in_=o1ps[:])
                    wwps = pgat.tile([NCH, NCH], F32, tag="pg")
                    nc.tensor.matmul(wwps[:], P(li, "wgT"), P(li, "wr"),
                                     start=True, stop=True)
                    WW = gat.tile([NCH, NCH], F32, tag="WW")
                    nc.vector.tensor_copy(out=WW[:], in_=wwps[:])
                    O1T = trans(O1[:], 20, NCH, "O1T")
                    o3ps = pgat.tile([NCH, 20], F32, tag="pg")
                    nc.tensor.matmul(o3ps[:], WW[:], O1T[0:NCH, 0:20],
                                     start=True, stop=True)
                    ce2 = gat.tile([NCH, 20], F32, tag="ce2")
                    nc.vector.scalar_tensor_tensor(
                        out=ce2[:], in0=o3ps[:], scalar=1.0, in1=O[:],
                        op0=ALU.add, op1=ALU.mult)
                    stats = gat.tile([NCH, nc.vector.BN_STATS_DIM], F32, tag="bnst")
                    nc.vector.bn_stats(out=stats[:], in_=ce2[:])
                    mv = gat.tile([NCH, nc.vector.BN_AGGR_DIM], F32, tag="bnmv")
                    nc.vector.bn_aggr(out=mv[:], in_=stats[:])
                    rstd = gat.tile([NCH, 1], F32, tag="rstd")
                    nc.scalar.activation(out=rstd[:], in_=mv[:, 1:2], func=AF.Sqrt,
                                         bias=epst[:], scale=1.0)
                    nc.vector.reciprocal(out=rstd[:], in_=rstd[:])
                    bn1 = gat.tile([NCH, 20], F32, tag="bn1")
                    nc.vector.tensor_scalar(
                        out=bn1[:], in0=ce2[:], scalar1=mv[:, 0:1], scalar2=rstd[:],
                        op0=ALU.subtract, op1=ALU.mult)
                    # t path
                    tr = gat.tile([NCH, 20], F32, tag="tr")
                    nc.vector.tensor_scalar(
                        out=tr[:], in0=bn1[:], scalar1=P(li, "bn")[:, 0:1],
                        scalar2=P(li, "bn")[:, 1:2], op0=ALU.mult, op1=ALU.add)
                    nc.vector.tensor_scalar_max(out=tr[:], in0=tr[:], scalar1=0.0)
                    tTs = trans(tr[:], NCH, 20, "tTs")
                    g1ps = pgat.tile([NCH, 36], F32, tag="pg")
                    nc.tensor.matmul(g1ps[:], tTs[0:20, 0:NCH], P(li, "gdBD"),
                                     start=True, stop=True)
                    G1 = gat.tile([NCH, 36], F32, tag="G1")
                    nc.vector.tensor_copy(out=G1[:], in_=g1ps[:])
                    # u path: block-diag ci matmul, no remaps
                    ur = gat.tile([NCH, 20], F32, tag="ur")
                    nc.vector.tensor_scalar(
                        out=ur[:], in0=bn1[:], scalar1=P(li, "bn")[:, 2:3],
                        scalar2=P(li, "bn")[:, 3:4], op0=ALU.mult, op1=ALU.add)
                    nc.vector.tensor_scalar_max(out=ur[:], in0=ur[:], scalar1=0.0)
                    u2ps = pgat.tile([NCH, 20], F32, tag="pg")
                    nc.tensor.matmul(u2ps[:], P(li, "ciBD"), ur[:],
                                     start=True, stop=True)
                    u3 = gat.tile([NCH, 20], F32, tag="u3")
                    nc.vector.tensor_copy(out=u3[:], in_=u2ps[:])
                    stats3 = gat.tile([NCH, nc.vector.BN_STATS_DIM], F32, tag="bnst3")
                    nc.vector.bn_stats(out=stats3[:], in_=u3[:])
                    mv3 = gat.tile([NCH, nc.vector.BN_AGGR_DIM], F32, tag="bnmv3")
                    nc.vector.bn_aggr(out=mv3[:], in_=stats3[:])
                    rstd3 = gat.tile([NCH, 1], F32, tag="rstd3")
                    nc.scalar.activation(out=rstd3[:], in_=mv3[:, 1:2], func=AF.Sqrt,
                                         bias=epst[:], scale=1.0)
                    nc.vector.reciprocal(out=rstd3[:], in_=rstd3[:])
                    uu = gat.tile([NCH, 20], F32, tag="uu")
                    nc.vector.tensor_scalar(
                        out=uu[:], in0=u3[:], scalar1=mv3[:, 0:1], scalar2=rstd3[:],
                        op0=ALU.subtract, op1=ALU.mult)
                    nc.vector.tensor_scalar(
                        out=uu[:], in0=uu[:], scalar1=P(li, "bn")[:, 4:5],
                        scalar2=P(li, "bn")[:, 5:6], op0=ALU.mult, op1=ALU.add)
                    nc.vector.tensor_scalar_max(out=uu[:], in0=uu[:], scalar1=0.0)
                    uTs = trans(uu[:], NCH, 20, "uTs")
                    g2ps = pgat.tile([NCH, 36], F32, tag="pg")
                    nc.tensor.matmul(g2ps[:], uTs[0:20, 0:NCH], P(li, "gd2BD"),
                                     start=True, stop=True)
                    G2 = gat.tile([NCH, 36], F32, tag="G2")
                    nc.vector.tensor_copy(out=G2[:], in_=g2ps[:])
                    # select this core's batch (q-ordered cols)
                    g1sel = gat.tile([NCH, 9], F32, tag="g1sel")
                    g2sel = gat.tile([NCH, 9], F32, tag="g2sel")
                    tmp = gat.tile([NCH, 36], F32, tag="gselt")
                    for gsel, Gt in ((g1sel, G1), (g2sel, G2)):
                        nc.vector.tensor_mul(out=tmp[:], in0=Gt[:], in1=bmaskt[:])
                        nc.vector.tensor_reduce(
                            out=ap_of(gsel, 0, NCH, 0, [[2, 3], [1, 2]]),
                            in_=ap_of(tmp, 0, NCH, 0, [[1, 3], [3, 2], [9, 4]]),
                            axis=mybir.AxisListType.X, op=ALU.add)
                        nc.vector.tensor_reduce(
                            out=ap_of(gsel, 0, NCH, 6, [[1, 3]]),
                            in_=ap_of(tmp, 0, NCH, 6, [[1, 3], [9, 4]]),
                            axis=mybir.AxisListType.X, op=ALU.add)
                    # broadcast g1sel over oc partitions (dram flat = q*64 + c)
                    g1b_d = dpool.tile([9, NCH], F32, tag="g1bd")
                    nc.sync.dma_start(
                        out=dram_ap(g1b_d, 0, [[1, NCH], [64, 9]]), in_=g1sel[:])
                    ksb = gat.tile([NCH, 576], F32, tag="ksb")
                    nc.sync.dma_start(
                        out=ksb[:], in_=dram_ap(g1b_d, 0, [[0, NCH], [1, 576]]))
                    nc.vector.tensor_tensor(
                        out=ksb[:], in0=ksb[:],
                        in1=ap_of(g2sel, 0, NCH, 0, [[1, 9], [0, NCH]]),
                        op=ALU.add)
                    nc.scalar.activation(out=ksb[:], in_=ksb[:], func=AF.Sigmoid)
                    nc.vector.tensor_mul(out=ksb[:], in0=ksb[:], in1=P(li, "wconvP"))
                    LP = per.tile([128, 192], F16, tag=f"LP{li}")
                    LS = per.tile([NCH, 192], F16, tag=f"LS{li}")
                    for p in range(3):
                        tps = pgat.tile([128, NCH], F32, tag="pg")
                        nc.tensor.transpose(
                            out=tps[:], in_=ksb[0:NCH, 128 * p:128 * p + 128],
                            identity=ident[:])
                        nc.vector.tensor_copy(out=LP[:, 64 * p:64 * p + 64], in_=tps[:])
                    for sq in range(3):
                        tss = pgat.tile([NCH, NCH], F32, tag="pg")
                        nc.tensor.transpose(
                            out=tss[:], in_=ksb[0:NCH, 384 + 64 * sq:448 + 64 * sq],
                            identity=ident[:])
                        nc.vector.tensor_copy(out=LS[:, 64 * sq:64 * sq + 64],
                                              in_=tss[:])
                    return LP, LS

                def conv_layer(src, LP, LS, lo, hi, shift, epilogue):
                    blocks = [(c0, min(CONV_BLK, hi - c0))
                              for c0 in range(lo, hi, CONV_BLK)]
                    nblk = 0
                    for pi in range(0, len(blocks), 2):
                        pair = blocks[pi:pi + 2]
                        ps = pconv.tile([128, CONV_BLK], F32, tag="convps")
                        for i, da in enumerate(PAIR_DELTAS):
                            for half, (c0, bw) in enumerate(pair):
                                nc.tensor.matmul(
                                    ps[64 * half:64 * half + NCH, 0:bw],
                                    LP[:, 64 * i:64 * i + 64],
                                    ap_of(src, 0, 128, XOFF + c0 + shift + da,
                                          [[1, bw]]),
                                    start=(i == 0), stop=False,
                                    tile_position=(0, 64 * half))
                        for j, d in enumerate(SINGLE_DELTAS):
                            for half, (c0, bw) in enumerate(pair):
                                nc.tensor.matmul(
                                    ps[64 * half:64 * half + NCH, 0:bw],
                                    LS[0:NCH, 64 * j:64 * j + 64],
                                    ap_of(src, 0, NCH, XOFF + c0 + shift + d,
                                          [[1, bw]]),
                                    start=False, stop=(j == 2),
                                    tile_position=(0, 64 * half))
                        for half, (c0, bw) in enumerate(pair):
                            epilogue(c0, bw, ps, 64 * half, nblk)
                            nblk += 1

                # ================= layer 1 =================
                glT1 = all_gather_gl(gl_reduce(xx, 3))
                LP1, LS1 = gating(1, glT1)

                def pad_fix(c0, bw):
                    # re-zero pad cols (both halves) in [c0-194, c0+bw)
                    first = (c0 - 194 - (XOFF + 193) + PW - 1) // PW
                    first = max(first, 0)
                    last = (c0 + bw - 1 - (XOFF + 193)) // PW
                    last = min(last, YSLOTS - 2)
                    if last >= first:
                        nc.gpsimd.memset(
                            ap_of(yy, 0, 128, XOFF + 193 + first * PW,
                                  [[PW, last - first + 1], [1, 2]]), 0.0)

                def epi1(c0, bw, ps, ph, nblk):
                    dst = ap_of(yy, 0, NCH, XOFF + c0, [[1, bw]])
                    if nblk % 2 == 0:
                        nc.scalar.activation(out=dst, in_=ps[ph:ph + NCH, 0:bw],
                                             func=AF.Relu)
                    else:
                        nc.vector.tensor_scalar_max(out=dst, in0=ps[ph:ph + NCH, 0:bw],
                                                    scalar1=0.0)
                    nc.sync.dma_start(
                        out=ap_of(yy, 64, 64, XOFF + c0 - 194, [[1, bw]]),
                        in_=ap_of(yy, 0, 64, XOFF + c0, [[1, bw]]))
                    pad_fix(c0, bw)

                conv_layer(xx, LP1, LS1, L1_LO, L1_HI, 194, epi1)

                # boundary row masks (slots 1 / 98 and their upper images)
                for part0 in (0, 64):
                    off = -194 if part0 else 0
                    for sl, col in ((0, XOFF + PW + off), (1, XOFF + 98 * PW + off)):
                        nc.vector.tensor_scalar_mul(
                            out=ap_of(yy, part0, 64, col, [[1, PW]]),
                            in0=ap_of(yy, part0, 64, col, [[1, PW]]),
                            scalar1=ymt[part0:part0 + 64, sl:sl + 1])

                # ================= layer 2 =================
                glT2 = all_gather_gl(gl_reduce(yy, 2))
                LP2, LS2 = gating(2, glT2)

                flush_state = {"rows": 0}

                def epi2(c0, bw, ps, ph, nblk):
                    nc.vector.tensor_tensor(
                        out=outt[0:NCH, c0 - L2_LO:c0 - L2_LO + bw],
                        in0=ps[ph:ph + NCH, 0:bw],
                        in1=ap_of(xx, 0, NCH, XOFF + c0 + 194, [[1, bw]]),
                        op=ALU.add)
                    done_rows = (c0 + bw - L2_LO) // PW
                    if done_rows - flush_state["rows"] >= 12 or (c0 + bw) == L2_HI:
                        r0, r1 = flush_state["rows"], done_rows
                        if (c0 + bw) == L2_HI:
                            r1 = 96
                        if r1 > r0:
                            nc.sync.dma_start(
                                out=out_d[:, r0:r1, :],
                                in_=ap_of(outt, 0, NCH, r0 * PW + 1,
                                          [[PW, r1 - r0], [1, 192]]))
                        flush_state["rows"] = r1

                conv_layer(yy, LP2, LS2, L2_LO, L2_HI, 0, epi2)
                if debug:
                    nc.sync.dma_start(out=dbg_d, in_=yy[:])

            if loop:
                with tc.For_i(0, loop, 1):
                    body()
            else:
                body()

    nc.compile()
    return nc


def host_prep(x, p1, p2):
    """Build per-core input maps from full inputs."""
    x = np.asarray(x, dtype=np.float32)
    packed = {}
    for li, p in ((1, p1), (2, p2)):
        p = {k: np.asarray(v, dtype=np.float32) for k, v in p.items()}
        buf = np.zeros((NCH, PCOLS), np.float32)
        eye4 = np.eye(4, dtype=np.float32)
        vals = dict(
            aff1T=p["aff1"].T, aff2T=p["aff2"].T, wgT=p["wg"].T, wr=p["wr"],
            wconvP=p["wconv"].reshape(NCH, NCH, 9)[:, :, KPERM]
                .transpose(0, 2, 1).reshape(NCH, 576),
            bn=np.stack([p["ce_bn_g"], p["ce_bn_b"], p["ci_bn2_g"],
                         p["ci_bn2_b"], p["ci_bn_g"], p["ci_bn_b"]], axis=1),
            ceBD=np.kron(eye4, p["ce"].T), gdBD=np.kron(eye4, p["gd"].T),
            gd2BD=np.kron(eye4, p["gd2"].T), ciBD=np.kron(eye4, p["ci"].T),
            atmask=np.kron(eye4, np.ones((5, 5), np.float32)))
        for name, (c0, (pp, w)) in POFF.items():
            buf[0:pp, c0:c0 + w] = vals[name]
        packed[li] = buf
    in_maps = []
    for core in range(N_CORES):
        b, h = core // 2, core % 2
        r0 = 96 * h
        sh = np.zeros((NCH, XSLOTS, PW), np.float16)
        lo, hi = r0 - 3, r0 + 99
        slo, shi = max(lo, 0), min(hi, 192)
        sh[:, slo - lo:shi - lo, 1:193] = x[b, :, slo:shi, :].astype(np.float16)
        sh = sh.reshape(NCH, XSLOTS * PW)
        m = {"xs": sh,
             "ym": np.array([[0.0], [1.0]] if h == 0 else [[1.0], [0.0]], np.float32),
             "bmask": np.kron(np.eye(4, dtype=np.float32)[b],
                              np.ones(9, np.float32)).reshape(1, 36),
             "pk_1": packed[1], "pk_2": packed[2]}
        in_maps.append(m)
    return in_maps


_CACHE = {}


def kernel(x, p1, p2):
    import concourse.bass_utils as bass_utils
    in_maps = host_prep(x, p1, p2)
    if "nc" not in _CACHE:
        _CACHE["nc"] = build_nc()
    nc = _CACHE["nc"]
    res = bass_utils.run_bass_kernel_spmd(
        nc, in_maps, core_ids=list(range(N_CORES)), trace=False)
    out = np.empty((4, NCH, 192, 192), np.float32)
    for core in range(N_CORES):
        b, h = core // 2, core % 2
        out[b, :, 96 * h:96 * h + 96, :] = res.results[core]["o"]
    return out
